# revision 16
# baseline (speedup 1.0000x reference)
"""GPTBigCode MQA causal attention block on 8 TRN2 NeuronCores.

Tensor-parallel over heads: each core computes 4 of 32 query heads (the
single KV head is replicated per the vLLM MQA convention), row-parallel
c_proj, partial outputs summed on host. A KV-sharded variant with AllGather
was measured and rejected: the mere presence of a collective in the NEFF
drops the PE to ~2.0 GHz for the whole run (~263ns vs ~216ns per N=512
matmul), costing more than the sharding saves.

All matmul operands are fp16 — same PE stream rate as fp32r (1 cycle/row at
N>=512) but LDWEIGHTS takes the fast-weight-load path, which matters because
this environment compiles with ldw-opt disabled and a serialized fp32 weight
load costs ~109ns per matmul. QKV is produced directly transposed
(qkv^T = W1^T X^T with X^T streamed as the moving operand) so Q^T and K^T
come out in attention layout with no PE transposes and no DRAM round trip;
only V needs 4 small transposes per chunk. Per 512-token chunk the emission
order is: attention (4 heads) -> QKV projection of the next chunk -> c_proj,
so the next chunk's projection matmuls cover the last head's normalization
latency and DMA stays overlapped throughout.

Softmax skips max-subtraction (unit-variance logits cannot overflow fp32).
Denominators come from an all-ones [128,128] fp16 stationary so they land
pre-broadcast across partitions ([128,q] PSUM) and normalization is a DVE
reciprocal+multiply with no PE involvement (an M=1 ones-column denominator
matmul measures ~350ns vs ~225ns for M=128, and the [1,q] result would need
a PE broadcast that stalls the array on the reciprocal). Causal masking adds
-1e30 to the triangular block of diagonal tiles; fully masked key blocks are
never computed.
"""

import numpy as np
from contextlib import ExitStack

import concourse.bass as bass
import concourse.tile as tile
from concourse import mybir
from concourse.bass_utils import run_bass_kernel_spmd
from concourse.masks import make_identity

B, S, D = 2, 2048, 4096
H, DH = 32, 128
P = 128
NCORES = 8
HC = H // NCORES          # 4 heads per core
DQC = HC * DH             # 512 q-dims per core
T = B * S                 # 4096 tokens
SCALE = DH ** -0.5
NKD = D // P              # 32 contraction tiles in model dim
E1 = DQC + 2 * DH         # 768 per-core QKV output dims
NE = E1 // P              # 6 e-tiles: 0-3 Q heads, 4 K, 5 V
QTILE = 512               # tokens per chunk
NCH = T // QTILE          # 8 chunks
NKT = S // P              # 16 key tiles per batch
CHT = QTILE // P          # 4 key tiles per chunk

F32 = mybir.dt.float32
F16 = mybir.dt.float16
ACTF = mybir.ActivationFunctionType
NEG = -1.0e30


def build_program():
    nc = bass.Bass()
    xt = nc.declare_dram_parameter("xt", [D, T], F16, isOutput=False)
    w1 = nc.declare_dram_parameter("w1", [D, E1], F16, isOutput=False)
    b1 = nc.declare_dram_parameter("b1", [P, NE], F32, isOutput=False)
    w2 = nc.declare_dram_parameter("w2", [DQC, D], F16, isOutput=False)
    maskp = nc.declare_dram_parameter("mask", [P, P], F32, isOutput=False)
    yt = nc.declare_dram_parameter("yt", [D, T], F16, isOutput=True)

    xt3 = xt.rearrange("(kd p) t -> p kd t", p=P)
    w13 = w1.rearrange("(kd p) e -> p kd e", p=P)
    w23 = w2.rearrange("(kh p) d -> p kh d", p=P)

    with tile.TileContext(nc) as tc:
        with ExitStack() as ctx:
            _body(ctx, tc, nc, xt3, w13, b1, w23, maskp, yt)
    _legalize_waits(nc)
    return nc


def _legalize_waits(nc, nop_cap=1):
    """walrus's per-instruction sync-wait budget is tiny for matmuls (LDW+MM
    lowering) and DMA pseudo-instructions. Drop redundant same-engine
    self-waits (engines execute in order), then spill excess waits onto
    same-engine NoOps inserted right before the instruction."""
    nocap = (mybir.InstNoOp,)
    f = nc.m.functions[0]
    for bb in f.blocks:
        insts = bb.instructions
        # pass 1: strip same-engine self-waits
        for i in insts:
            si = i.sync_info
            if si is None or not si.on_wait:
                continue
            ename = str(i.engine).split(".")[-1]
            if ename == "SP":
                ename = "Sync"
            kept = [w for w in si.on_wait
                    if w.sync_type != "semaphore"
                    or w.wait_reg is not None
                    or not w.ant_name.split("_")[0] == ename]
            if len(kept) != len(si.on_wait):
                si.on_wait = kept
        # pass 2: spill excess waits onto preceding nops
        idx = 0
        while idx < len(insts):
            i = insts[idx]
            si = i.sync_info
            cap = None if isinstance(i, nocap) else 1
            if cap is not None and si is not None and len(si.on_wait) > cap:
                excess = list(si.on_wait[:-cap])
                si.on_wait = list(si.on_wait[-cap:])
                while excess:
                    chunk, excess = excess[:nop_cap], excess[nop_cap:]
                    nop = mybir.InstNoOp(
                        name=nc.get_next_instruction_name(), ins=[], outs=[])
                    nop.engine = i.engine
                    nop.sync_info = mybir.SyncInfo(on_wait=chunk, on_update=[])
                    nc.register_instruction(nop)
                    insts.insert(idx, nop)
                    idx += 1
            idx += 1


class _QFeeder:
    """Emits the Q-projection matmuls for one chunk a few at a time, so they
    can be slotted between attention matmuls as latency filler. One e-tile
    (32 accumulating matmuls + eviction) per attention head works out
    exactly: 4 e-tiles, 4 heads."""

    def __init__(self, st, tcn):
        self.st = st
        self.tcn = tcn
        nc = st["nc"]
        self.xc = st["xc_pool"].tile([P, NKD, QTILE], F16, tag="xc",
                                     name=f"xc{tcn}")
        nc.sync.dma_start(
            out=self.xc[:],
            in_=st["xt3"][:, :, tcn * QTILE:(tcn + 1) * QTILE])
        self.qt = st["qt_pool"].tile([P, HC, QTILE], F16, tag="qt",
                                     name=f"qt{tcn}")
        self.e = 0
        self.kd = 0
        self.ps = None

    def feed(self, n):
        st = self.st
        nc = st["nc"]
        tcn = self.tcn
        order = (4, 5, 0, 1, 2, 3)
        for _ in range(n):
            if self.e >= NE:
                return
            e = order[self.e]
            if self.ps is None:
                self.ps = st["ps_mm"].tile([P, QTILE], F32, tag="mm",
                                           name=f"psq{tcn}_{e}")
            nc.tensor.matmul(
                self.ps[:],
                st["w1_sb"][:, self.kd, e * P:(e + 1) * P],
                self.xc[:, self.kd, :],
                start=(self.kd == 0), stop=(self.kd == NKD - 1))
            self.kd += 1
            if self.kd == NKD:
                if e < HC:
                    nc.scalar.activation(self.qt[:, e, :], self.ps[:],
                                         ACTF.Identity,
                                         bias=st["b1_sb"][:, e:e + 1])
                elif e == 4:
                    nc.scalar.activation(
                        st["kt_sb"][:, tcn * QTILE:(tcn + 1) * QTILE],
                        self.ps[:], ACTF.Identity,
                        bias=st["b1_sb"][:, e:e + 1])
                else:
                    vv = st["vv_pool"].tile([P, QTILE], F16, tag="vv",
                                            name=f"vv{tcn}")
                    nc.scalar.activation(vv[:], self.ps[:], ACTF.Identity,
                                         bias=st["b1_sb"][:, e:e + 1])
                    for r in range(CHT):
                        tp = st["ps_aux"].tile([P, P], F16, tag="aux",
                                               name=f"tp{tcn}_{r}")
                        nc.tensor.transpose(tp[:], vv[:, r * P:(r + 1) * P],
                                            st["ident"][:])
                        nc.scalar.activation(v_sb_slot(st, tcn, r), tp[:],
                                             ACTF.Copy)
                self.ps = None
                self.kd = 0
                self.e += 1

    def flush(self):
        self.feed(NE * NKD)
        return self.qt


def v_sb_slot(st, tcn, r):
    return st["v_sb"][:, tcn * CHT + r, :]


def _attention(st, tcn, qt_t, feeder):
    """4 heads of causal MQA for one 512-token chunk, feeder matmuls
    interleaved as latency filler. Returns the 4 normalized at tiles."""
    nc = st["nc"]
    b, j = divmod(tcn, NCH // B)
    nk = CHT * j + CHT
    at_tiles = []
    for h in range(HC):
        ps_out = st["ps_acc"].tile([P, QTILE], F32, tag="acc",
                                   name=f"po{tcn}_{h}")
        ps_den = st["ps_aux"].tile([P, QTILE], F32, tag="aux",
                                   name=f"pd{tcn}_{h}")
        for kk in range(nk):
            r = kk - CHT * j
            qoff = 0 if r < 0 else P * r
            c0 = b * S + kk * P
            p_t = st["p_pool"].tile([P, QTILE], F16, tag="p",
                                    name=f"pt{tcn}_{h}_{kk}")
            ps_s = st["ps_mm"].tile([P, QTILE], F32, tag="mm",
                                    name=f"pss{tcn}_{h}_{kk}")
            nc.tensor.matmul(ps_s[:, qoff:], st["kt_sb"][:, c0:c0 + P],
                             qt_t[:, h, qoff:], start=True, stop=True)
            if r >= 0:
                nc.vector.tensor_add(ps_s[:, qoff:qoff + P],
                                     ps_s[:, qoff:qoff + P], st["mask_sb"][:])
            nc.scalar.activation(p_t[:, qoff:], ps_s[:, qoff:],
                                 ACTF.Exp, scale=SCALE)
            if feeder is not None:
                feeder.feed(max(2, NKD // nk))
            nc.tensor.matmul(ps_out[:, qoff:], st["v_sb"][:, b * NKT + kk, :],
                             p_t[:, qoff:],
                             start=(kk == 0), stop=(kk == nk - 1))
            nc.tensor.matmul(ps_den[:, qoff:], st["ones_pp"][:],
                             p_t[:, qoff:],
                             start=(kk == 0), stop=(kk == nk - 1))
        inv_bc = st["p_pool"].tile([P, QTILE], F32, tag="invbc",
                                   name=f"inv{tcn}_{h}")
        nc.vector.reciprocal(inv_bc[:], ps_den[:])
        at_t = st["at_pool"].tile([P, QTILE], F16, tag="at",
                                  name=f"at{tcn}_{h}")
        nc.vector.tensor_mul(at_t[:], ps_out[:], inv_bc[:])
        at_tiles.append(at_t)
    return at_tiles


def _c_proj(st, tcn, at_tiles):
    """y^T[dout, t] += W2h^T at_h^T. The first 3 me-tiles run kh 0-2 up
    front so the tensor engine has at[3]-independent work while head 3's
    normalization chain drains."""
    nc = st["nc"]
    t0 = tcn * QTILE
    ps_ys = [st["ps_acc"].tile([P, QTILE], F32, tag="acc", name=f"py{i}")
             for i in range(3)]
    for me in range(3):
        for kh in range(3):
            nc.tensor.matmul(ps_ys[me][:],
                             st["w2_sb"][:, kh, me * P:(me + 1) * P],
                             at_tiles[kh][:], start=(kh == 0), stop=False)
    for me in range(D // P):
        if me < 3:
            ps_y = ps_ys[me]
            nc.tensor.matmul(ps_y[:], st["w2_sb"][:, 3, me * P:(me + 1) * P],
                             at_tiles[3][:], start=False, stop=True)
        else:
            ps_y = st["ps_acc"].tile([P, QTILE], F32, tag="acc",
                                     name=f"py{tcn}_{me}")
            for kh in range(HC):
                nc.tensor.matmul(ps_y[:],
                                 st["w2_sb"][:, kh, me * P:(me + 1) * P],
                                 at_tiles[kh][:],
                                 start=(kh == 0), stop=(kh == HC - 1))
        y_t = st["y_pool"].tile([P, QTILE], F16, tag="y", name=f"y{tcn}_{me}")
        nc.scalar.activation(y_t[:], ps_y[:], ACTF.Copy)
        nc.sync.dma_start(out=st["yt"][me * P:(me + 1) * P, t0:t0 + QTILE],
                          in_=y_t[:])


def _body(ctx, tc, nc, xt3, w13, b1, w23, maskp, yt):
    persist = ctx.enter_context(tc.tile_pool(name="persist", bufs=1))
    w1_sb = persist.tile([P, NKD, E1], F16)      # qkv weights, lhsT tiles
    w2_sb = persist.tile([P, HC, D], F16)        # c_proj weights, lhsT tiles
    b1_sb = persist.tile([P, NE], F32)
    kt_sb = persist.tile([P, T], F16)            # K^T [dh, t]
    v_sb = persist.tile([P, T // P, DH], F16)    # V   [t_part, mt, dh]
    ones_pp = persist.tile([P, P], F16)          # den stationary (K=P, M=P)
    mask_sb = persist.tile([P, P], F32)          # additive causal mask
    ident = persist.tile([P, P], F16)

    nc.sync.dma_start(out=w1_sb[:], in_=w13[:])
    nc.sync.dma_start(out=w2_sb[:], in_=w23[:])
    nc.sync.dma_start(out=b1_sb[:], in_=b1[:])
    nc.vector.memset(ones_pp[:], 1.0)
    nc.sync.dma_start(out=mask_sb[:], in_=maskp[:])
    make_identity(nc, ident[:])

    # PSUM pools: 3 + 3 + 2 banks = 8
    ps_mm = ctx.enter_context(tc.tile_pool(name="ps_mm", bufs=3, space="PSUM"))
    ps_acc = ctx.enter_context(tc.tile_pool(name="ps_acc", bufs=3, space="PSUM"))
    ps_aux = ctx.enter_context(tc.tile_pool(name="ps_aux", bufs=2, space="PSUM"))

    st = {
        "nc": nc, "xt3": xt3, "yt": yt,
        "w1_sb": w1_sb, "w2_sb": w2_sb, "b1_sb": b1_sb,
        "kt_sb": kt_sb, "v_sb": v_sb, "ones_pp": ones_pp,
        "mask_sb": mask_sb, "ident": ident,
        "ps_mm": ps_mm, "ps_acc": ps_acc, "ps_aux": ps_aux,
        "xc_pool": ctx.enter_context(tc.tile_pool(name="xc", bufs=2)),
        "qt_pool": ctx.enter_context(tc.tile_pool(name="qt", bufs=2)),
        "vv_pool": ctx.enter_context(tc.tile_pool(name="vv", bufs=2)),
        "misc_pool": ctx.enter_context(tc.tile_pool(name="misc", bufs=1)),
        "p_pool": ctx.enter_context(tc.tile_pool(name="pp", bufs=5)),
        "at_pool": ctx.enter_context(tc.tile_pool(name="at", bufs=8)),
        "y_pool": ctx.enter_context(tc.tile_pool(name="yp", bufs=3)),
        "dram": ctx.enter_context(tc.tile_pool(name="dram", bufs=1,
                                               space="DRAM")),
    }

    qts = {0: _QFeeder(st, 0).flush()}

    # Next-chunk feeders are created (allocating the tile and emitting the
    # X^T DMA) just before c_proj of the previous chunk, so the 4MB load
    # always has the whole c_proj plus the next attention as cover.
    feeder = _QFeeder(st, 1)
    for tcn in range(NCH):
        at_tiles = _attention(st, tcn, qts[tcn], None)
        if feeder is not None:
            qts[tcn + 1] = feeder.flush()
        feeder = _QFeeder(st, tcn + 2) if tcn + 2 < NCH else None
        _c_proj(st, tcn, at_tiles)


_PROGRAM = None


def _get_program():
    global _PROGRAM
    if _PROGRAM is None:
        _PROGRAM = build_program()
    return _PROGRAM


def make_in_maps(hidden_states, w_qkv, b_qkv, w_proj, b_proj):
    x = np.asarray(hidden_states, dtype=np.float32).reshape(T, D)
    xt = np.ascontiguousarray(x.T.astype(np.float16))
    ki = np.arange(P)[:, None]
    qj = np.arange(P)[None, :]
    mask = np.where(ki <= qj, 0.0, NEG).astype(np.float32)
    w_qkv = np.asarray(w_qkv, dtype=np.float32)
    b_qkv = np.asarray(b_qkv, dtype=np.float32)
    w_proj = np.asarray(w_proj, dtype=np.float32)
    in_maps = []
    for c in range(NCORES):
        qcols = slice(c * DQC, (c + 1) * DQC)
        w1 = np.concatenate([w_qkv[:, qcols], w_qkv[:, D:]], axis=1)
        b1 = np.concatenate([b_qkv[qcols], b_qkv[D:]])
        in_maps.append({
            "xt": xt,
            "w1": np.ascontiguousarray(w1.astype(np.float16)),
            "b1": np.ascontiguousarray(b1.reshape(NE, P).T.astype(np.float32)),
            "w2": np.ascontiguousarray(
                w_proj[c * DQC:(c + 1) * DQC, :].astype(np.float16)),
            "mask": mask,
        })
    return in_maps


def kernel(hidden_states, w_qkv, b_qkv, w_proj, b_proj):
    nc = _get_program()
    in_maps = make_in_maps(hidden_states, w_qkv, b_qkv, w_proj, b_proj)
    res = run_bass_kernel_spmd(nc, in_maps, list(range(NCORES)))
    y = np.zeros((D, T), dtype=np.float32)
    for r in res.results:
        y += np.asarray(r["yt"], dtype=np.float32)
    y = y.T + np.asarray(b_proj, dtype=np.float32)[None, :]
    return np.ascontiguousarray(y.reshape(B, S, D)).astype(np.float32)


# revision 17
# speedup vs baseline: 1.0187x; 1.0187x over previous
"""GPTBigCode MQA causal attention block on 8 TRN2 NeuronCores.

Tensor-parallel over heads: each core computes 4 of 32 query heads (the
single KV head is replicated per the vLLM MQA convention), row-parallel
c_proj, partial outputs summed on host. A KV-sharded variant with AllGather
was measured and rejected: the mere presence of a collective in the NEFF
drops the PE to ~2.0 GHz for the whole run (~263ns vs ~216ns per N=512
matmul), costing more than the sharding saves.

All matmul operands are fp16 — same PE stream rate as fp32r (1 cycle/row at
N>=512) but LDWEIGHTS takes the fast-weight-load path, which matters because
this environment compiles with ldw-opt disabled and a serialized fp32 weight
load costs ~109ns per matmul. QKV is produced directly transposed
(qkv^T = W1^T X^T with X^T streamed as the moving operand) so Q^T and K^T
come out in attention layout with no PE transposes and no DRAM round trip;
only V needs 4 small transposes per chunk. Per 512-token chunk the emission
order is: attention (4 heads) -> QKV projection of the next chunk -> c_proj,
so the next chunk's projection matmuls cover the last head's normalization
latency and DMA stays overlapped throughout.

Softmax skips max-subtraction (unit-variance logits cannot overflow fp32).
Denominators come from an all-ones [128,128] fp16 stationary so they land
pre-broadcast across partitions ([128,q] PSUM) and normalization is a DVE
reciprocal+multiply with no PE involvement (an M=1 ones-column denominator
matmul measures ~350ns vs ~225ns for M=128, and the [1,q] result would need
a PE broadcast that stalls the array on the reciprocal). Causal masking adds
-1e30 to the triangular block of diagonal tiles; fully masked key blocks are
never computed.
"""

import numpy as np
from contextlib import ExitStack

import concourse.bass as bass
import concourse.tile as tile
from concourse import mybir
from concourse.bass_utils import run_bass_kernel_spmd
from concourse.masks import make_identity

B, S, D = 2, 2048, 4096
H, DH = 32, 128
P = 128
NCORES = 8
HC = H // NCORES          # 4 heads per core
DQC = HC * DH             # 512 q-dims per core
T = B * S                 # 4096 tokens
SCALE = DH ** -0.5
NKD = D // P              # 32 contraction tiles in model dim
E1 = DQC + 2 * DH         # 768 per-core QKV output dims
NE = E1 // P              # 6 e-tiles: 0-3 Q heads, 4 K, 5 V
QTILE = 512               # tokens per chunk
NCH = T // QTILE          # 8 chunks
NKT = S // P              # 16 key tiles per batch
CHT = QTILE // P          # 4 key tiles per chunk

F32 = mybir.dt.float32
F16 = mybir.dt.float16
ACTF = mybir.ActivationFunctionType
NEG = -1.0e30


def build_program():
    nc = bass.Bass()
    # xt/w1/w2 arrive pre-tiled from the host (partition dim second/first,
    # contiguous within each partition row) so every DMA is one contiguous
    # segment per partition and stays on the HW descriptor-generation path.
    # The naive [D, T] slice pattern fell back to Sync-engine DIRECT2D
    # descriptor generation: ~4096 descriptors x 42ns = 8.7us per chunk.
    xt = nc.declare_dram_parameter("xt", [NCH, P, NKD, QTILE], F16,
                                   isOutput=False)
    w1 = nc.declare_dram_parameter("w1", [P, NKD, E1], F16, isOutput=False)
    b1 = nc.declare_dram_parameter("b1", [P, NE], F32, isOutput=False)
    w2 = nc.declare_dram_parameter("w2", [P, HC, D], F16, isOutput=False)
    maskp = nc.declare_dram_parameter("mask", [P, P], F32, isOutput=False)
    yt = nc.declare_dram_parameter("yt", [D, T], F16, isOutput=True)

    with tile.TileContext(nc) as tc:
        with ExitStack() as ctx:
            _body(ctx, tc, nc, xt, w1, b1, w2, maskp, yt)
    _legalize_waits(nc)
    return nc


def _legalize_waits(nc, nop_cap=1):
    """walrus's per-instruction sync-wait budget is tiny for matmuls (LDW+MM
    lowering) and DMA pseudo-instructions. Drop redundant same-engine
    self-waits (engines execute in order), then spill excess waits onto
    same-engine NoOps inserted right before the instruction."""
    nocap = (mybir.InstNoOp,)
    f = nc.m.functions[0]
    for bb in f.blocks:
        insts = bb.instructions
        # pass 1: strip same-engine self-waits
        for i in insts:
            si = i.sync_info
            if si is None or not si.on_wait:
                continue
            ename = str(i.engine).split(".")[-1]
            if ename == "SP":
                ename = "Sync"
            kept = [w for w in si.on_wait
                    if w.sync_type != "semaphore"
                    or w.wait_reg is not None
                    or not w.ant_name.split("_")[0] == ename]
            if len(kept) != len(si.on_wait):
                si.on_wait = kept
        # pass 2: spill excess waits onto preceding nops
        idx = 0
        while idx < len(insts):
            i = insts[idx]
            si = i.sync_info
            cap = None if isinstance(i, nocap) else 1
            if cap is not None and si is not None and len(si.on_wait) > cap:
                excess = list(si.on_wait[:-cap])
                si.on_wait = list(si.on_wait[-cap:])
                while excess:
                    chunk, excess = excess[:nop_cap], excess[nop_cap:]
                    nop = mybir.InstNoOp(
                        name=nc.get_next_instruction_name(), ins=[], outs=[])
                    nop.engine = i.engine
                    nop.sync_info = mybir.SyncInfo(on_wait=chunk, on_update=[])
                    nc.register_instruction(nop)
                    insts.insert(idx, nop)
                    idx += 1
            idx += 1


class _QFeeder:
    """Emits the Q-projection matmuls for one chunk a few at a time, so they
    can be slotted between attention matmuls as latency filler. One e-tile
    (32 accumulating matmuls + eviction) per attention head works out
    exactly: 4 e-tiles, 4 heads."""

    def __init__(self, st, tcn):
        self.st = st
        self.tcn = tcn
        nc = st["nc"]
        self.xc = st["xc_pool"].tile([P, NKD, QTILE], F16, tag="xc",
                                     name=f"xc{tcn}")
        nc.sync.dma_start(out=self.xc[:, 0:NKD // 2, :],
                          in_=st["xt4"][tcn, :, 0:NKD // 2, :])
        nc.sync.dma_start(out=self.xc[:, NKD // 2:, :],
                          in_=st["xt4"][tcn, :, NKD // 2:, :])
        self.qt = st["qt_pool"].tile([P, HC, QTILE], F16, tag="qt",
                                     name=f"qt{tcn}")
        self.e = 0
        self.kd = 0
        self.ps = None

    def feed(self, n):
        st = self.st
        nc = st["nc"]
        tcn = self.tcn
        order = (4, 5, 0, 1, 2, 3)
        for _ in range(n):
            if self.e >= NE:
                return
            e = order[self.e]
            if self.ps is None:
                self.ps = st["ps_mm"].tile([P, QTILE], F32, tag="mm",
                                           name=f"psq{tcn}_{e}")
            nc.tensor.matmul(
                self.ps[:],
                st["w1_sb"][:, self.kd, e * P:(e + 1) * P],
                self.xc[:, self.kd, :],
                start=(self.kd == 0), stop=(self.kd == NKD - 1))
            self.kd += 1
            if self.kd == NKD:
                if e < HC:
                    nc.scalar.activation(self.qt[:, e, :], self.ps[:],
                                         ACTF.Identity,
                                         bias=st["b1_sb"][:, e:e + 1])
                elif e == 4:
                    nc.scalar.activation(
                        st["kt_sb"][:, tcn * QTILE:(tcn + 1) * QTILE],
                        self.ps[:], ACTF.Identity,
                        bias=st["b1_sb"][:, e:e + 1])
                else:
                    vv = st["vv_pool"].tile([P, QTILE], F16, tag="vv",
                                            name=f"vv{tcn}")
                    nc.scalar.activation(vv[:], self.ps[:], ACTF.Identity,
                                         bias=st["b1_sb"][:, e:e + 1])
                    for r in range(CHT):
                        tp = st["ps_aux"].tile([P, P], F16, tag="aux",
                                               name=f"tp{tcn}_{r}")
                        nc.tensor.transpose(tp[:], vv[:, r * P:(r + 1) * P],
                                            st["ident"][:])
                        nc.scalar.activation(v_sb_slot(st, tcn, r), tp[:],
                                             ACTF.Copy)
                self.ps = None
                self.kd = 0
                self.e += 1

    def flush(self):
        self.feed(NE * NKD)
        return self.qt


def v_sb_slot(st, tcn, r):
    return st["v_sb"][:, tcn * CHT + r, :]


def _attention(st, tcn, qt_t, feeder):
    """4 heads of causal MQA for one 512-token chunk, feeder matmuls
    interleaved as latency filler. Returns the 4 normalized at tiles."""
    nc = st["nc"]
    b, j = divmod(tcn, NCH // B)
    nk = CHT * j + CHT
    at_tiles = []
    for h in range(HC):
        ps_out = st["ps_acc"].tile([P, QTILE], F32, tag="acc",
                                   name=f"po{tcn}_{h}")
        ps_den = st["ps_aux"].tile([P, QTILE], F32, tag="aux",
                                   name=f"pd{tcn}_{h}")
        for kk in range(nk):
            r = kk - CHT * j
            qoff = 0 if r < 0 else P * r
            c0 = b * S + kk * P
            p_t = st["p_pool"].tile([P, QTILE], F16, tag="p",
                                    name=f"pt{tcn}_{h}_{kk}")
            ps_s = st["ps_mm"].tile([P, QTILE], F32, tag="mm",
                                    name=f"pss{tcn}_{h}_{kk}")
            nc.tensor.matmul(ps_s[:, qoff:], st["kt_sb"][:, c0:c0 + P],
                             qt_t[:, h, qoff:], start=True, stop=True)
            if r >= 0:
                nc.vector.tensor_add(ps_s[:, qoff:qoff + P],
                                     ps_s[:, qoff:qoff + P], st["mask_sb"][:])
            nc.scalar.activation(p_t[:, qoff:], ps_s[:, qoff:],
                                 ACTF.Exp, scale=SCALE)
            if feeder is not None:
                feeder.feed(max(2, NKD // nk))
            nc.tensor.matmul(ps_out[:, qoff:], st["v_sb"][:, b * NKT + kk, :],
                             p_t[:, qoff:],
                             start=(kk == 0), stop=(kk == nk - 1))
            nc.tensor.matmul(ps_den[:, qoff:], st["ones_pp"][:],
                             p_t[:, qoff:],
                             start=(kk == 0), stop=(kk == nk - 1))
        inv_bc = st["p_pool"].tile([P, QTILE], F32, tag="invbc",
                                   name=f"inv{tcn}_{h}")
        nc.vector.reciprocal(inv_bc[:], ps_den[:])
        at_t = st["at_pool"].tile([P, QTILE], F16, tag="at",
                                  name=f"at{tcn}_{h}")
        nc.vector.tensor_mul(at_t[:], ps_out[:], inv_bc[:])
        at_tiles.append(at_t)
    return at_tiles


def _c_proj(st, tcn, at_tiles):
    """y^T[dout, t] += W2h^T at_h^T. The first 3 me-tiles run kh 0-2 up
    front so the tensor engine has at[3]-independent work while head 3's
    normalization chain drains."""
    nc = st["nc"]
    t0 = tcn * QTILE
    ps_ys = [st["ps_acc"].tile([P, QTILE], F32, tag="acc", name=f"py{i}")
             for i in range(3)]
    for me in range(3):
        for kh in range(3):
            nc.tensor.matmul(ps_ys[me][:],
                             st["w2_sb"][:, kh, me * P:(me + 1) * P],
                             at_tiles[kh][:], start=(kh == 0), stop=False)
    for me in range(D // P):
        if me < 3:
            ps_y = ps_ys[me]
            nc.tensor.matmul(ps_y[:], st["w2_sb"][:, 3, me * P:(me + 1) * P],
                             at_tiles[3][:], start=False, stop=True)
        else:
            ps_y = st["ps_acc"].tile([P, QTILE], F32, tag="acc",
                                     name=f"py{tcn}_{me}")
            for kh in range(HC):
                nc.tensor.matmul(ps_y[:],
                                 st["w2_sb"][:, kh, me * P:(me + 1) * P],
                                 at_tiles[kh][:],
                                 start=(kh == 0), stop=(kh == HC - 1))
        y_t = st["y_pool"].tile([P, QTILE], F16, tag="y", name=f"y{tcn}_{me}")
        nc.scalar.activation(y_t[:], ps_y[:], ACTF.Copy)
        nc.sync.dma_start(out=st["yt"][me * P:(me + 1) * P, t0:t0 + QTILE],
                          in_=y_t[:])


def _body(ctx, tc, nc, xt4, w14, b1, w24, maskp, yt):
    persist = ctx.enter_context(tc.tile_pool(name="persist", bufs=1))
    w1_sb = persist.tile([P, NKD, E1], F16)      # qkv weights, lhsT tiles
    w2_sb = persist.tile([P, HC, D], F16)        # c_proj weights, lhsT tiles
    b1_sb = persist.tile([P, NE], F32)
    kt_sb = persist.tile([P, T], F16)            # K^T [dh, t]
    v_sb = persist.tile([P, T // P, DH], F16)    # V   [t_part, mt, dh]
    ones_pp = persist.tile([P, P], F16)          # den stationary (K=P, M=P)
    mask_sb = persist.tile([P, P], F32)          # additive causal mask
    ident = persist.tile([P, P], F16)

    # halves so the first projection matmuls wait on 3MB, not 6MB
    nc.sync.dma_start(out=w1_sb[:, 0:NKD // 2, :], in_=w14[:, 0:NKD // 2, :])
    nc.sync.dma_start(out=w1_sb[:, NKD // 2:, :], in_=w14[:, NKD // 2:, :])
    nc.sync.dma_start(out=w2_sb[:], in_=w24[:])
    nc.sync.dma_start(out=b1_sb[:], in_=b1[:])
    nc.vector.memset(ones_pp[:], 1.0)
    nc.sync.dma_start(out=mask_sb[:], in_=maskp[:])
    make_identity(nc, ident[:])

    # PSUM pools: 3 + 3 + 2 banks = 8
    ps_mm = ctx.enter_context(tc.tile_pool(name="ps_mm", bufs=3, space="PSUM"))
    ps_acc = ctx.enter_context(tc.tile_pool(name="ps_acc", bufs=3, space="PSUM"))
    ps_aux = ctx.enter_context(tc.tile_pool(name="ps_aux", bufs=2, space="PSUM"))

    st = {
        "nc": nc, "xt4": xt4, "yt": yt,
        "w1_sb": w1_sb, "w2_sb": w2_sb, "b1_sb": b1_sb,
        "kt_sb": kt_sb, "v_sb": v_sb, "ones_pp": ones_pp,
        "mask_sb": mask_sb, "ident": ident,
        "ps_mm": ps_mm, "ps_acc": ps_acc, "ps_aux": ps_aux,
        "xc_pool": ctx.enter_context(tc.tile_pool(name="xc", bufs=2)),
        "qt_pool": ctx.enter_context(tc.tile_pool(name="qt", bufs=2)),
        "vv_pool": ctx.enter_context(tc.tile_pool(name="vv", bufs=2)),
        "misc_pool": ctx.enter_context(tc.tile_pool(name="misc", bufs=1)),
        "p_pool": ctx.enter_context(tc.tile_pool(name="pp", bufs=5)),
        "at_pool": ctx.enter_context(tc.tile_pool(name="at", bufs=8)),
        "y_pool": ctx.enter_context(tc.tile_pool(name="yp", bufs=3)),
        "dram": ctx.enter_context(tc.tile_pool(name="dram", bufs=1,
                                               space="DRAM")),
    }

    qts = {0: _QFeeder(st, 0).flush()}

    # Next-chunk feeders are created (allocating the tile and emitting the
    # X^T DMA) just before c_proj of the previous chunk, so the 4MB load
    # always has the whole c_proj plus the next attention as cover.
    feeder = _QFeeder(st, 1)
    for tcn in range(NCH):
        at_tiles = _attention(st, tcn, qts[tcn], None)
        if feeder is not None:
            qts[tcn + 1] = feeder.flush()
        feeder = _QFeeder(st, tcn + 2) if tcn + 2 < NCH else None
        _c_proj(st, tcn, at_tiles)


_PROGRAM = None


def _get_program():
    global _PROGRAM
    if _PROGRAM is None:
        _PROGRAM = build_program()
    return _PROGRAM


def make_in_maps(hidden_states, w_qkv, b_qkv, w_proj, b_proj):
    x = np.asarray(hidden_states, dtype=np.float32).reshape(T, D)
    xt = np.ascontiguousarray(
        x.T.astype(np.float16).reshape(NKD, P, NCH, QTILE)
        .transpose(2, 1, 0, 3))
    ki = np.arange(P)[:, None]
    qj = np.arange(P)[None, :]
    mask = np.where(ki <= qj, 0.0, NEG).astype(np.float32)
    w_qkv = np.asarray(w_qkv, dtype=np.float32)
    b_qkv = np.asarray(b_qkv, dtype=np.float32)
    w_proj = np.asarray(w_proj, dtype=np.float32)
    in_maps = []
    for c in range(NCORES):
        qcols = slice(c * DQC, (c + 1) * DQC)
        w1 = np.concatenate([w_qkv[:, qcols], w_qkv[:, D:]], axis=1)
        b1 = np.concatenate([b_qkv[qcols], b_qkv[D:]])
        in_maps.append({
            "xt": xt,
            "w1": np.ascontiguousarray(
                w1.astype(np.float16).reshape(NKD, P, E1).transpose(1, 0, 2)),
            "b1": np.ascontiguousarray(b1.reshape(NE, P).T.astype(np.float32)),
            "w2": np.ascontiguousarray(
                w_proj[c * DQC:(c + 1) * DQC, :].astype(np.float16)
                .reshape(HC, P, D).transpose(1, 0, 2)),
            "mask": mask,
        })
    return in_maps


def kernel(hidden_states, w_qkv, b_qkv, w_proj, b_proj):
    nc = _get_program()
    in_maps = make_in_maps(hidden_states, w_qkv, b_qkv, w_proj, b_proj)
    res = run_bass_kernel_spmd(nc, in_maps, list(range(NCORES)))
    y = np.zeros((D, T), dtype=np.float32)
    for r in res.results:
        y += np.asarray(r["yt"], dtype=np.float32)
    y = y.T + np.asarray(b_proj, dtype=np.float32)[None, :]
    return np.ascontiguousarray(y.reshape(B, S, D)).astype(np.float32)
